# revision 25
# baseline (speedup 1.0000x reference)
"""Trainium2 Bass kernel for GQA attention (B=2, S=2048, D=2048, H=32, KVH=8).

Sharding: 8 cores = 2 batches x 4 head-groups. Each core handles one batch and
8 q-heads / 2 kv-heads: wq/wk/wv column-parallel, wo row-parallel; the partial
wo products are summed on the host.

Host-side prep (pure layout, no math): inputs are sharded, head-permuted and
pre-transposed so every matmul operand DMAs straight into its [K-on-partition]
layout; cos/sin of the rope angles are also computed host-side (the ScalarE Sin
LUT only covers [-pi, pi]).

Per-core kernel (all matmuls bf16/f32r):
  - q/k/v projections computed with s on partitions ([s,o] layout) from the
    pre-transposed xT/wqT/wkvT, RoPE applied with strided DVE ops writing bf16,
    then q/k transposed to [o,s] via DMA xbar transpose (off the PE).
  - scores are computed transposed: scT[k,q] = kT.T @ qT per head; exp on ACT;
    causal handled by skipping fully-masked k-tiles + affine_select on diagonal.
  - PV: per (pair-group, pair) one [65,1024] PSUM accumulator; lhsT =
    [v_head | ones] (M=65) so the softmax denominator accumulates in PSUM row
    64 alongside the output. Zinv = exp(-ln Z) on the whole [1,1024] row in two
    ACT ops (ln+exp share one ACT table set with the scores' exp -> no ACT
    table reloads anywhere). Zinv broadcast across partitions via K=1 PE
    matmuls; attnT = outT * Zinv on DVE.
  - final: res[s,d] = sum_p attnT_p.T @ woT_p, accumulated over 4 o-blocks;
    PSUM->SBUF evacuation on DVE.

Head order within a core is permuted to [0,4,1,5,2,6,3,7] so that each
128-partition block pairs head h (kv0) with h+4 (kv1), letting the K=64 score
matmuls row-pack two heads concurrently on the PE array.

PSUM budget (8 banks): scores [128,1024] x2 bufs = 4, PV [65,1024] x1 = 2,
shared proj/final/broadcast [.,512] x2 = 2 - so projection and final-matmul
work can overlap the ACT-bound attention inner loops.
"""

import os
import sys

for _p in ("/opt/trn_rl_repo", "/root/.axon_site/_ro/trn_rl_repo"):
    if os.path.isdir(_p) and _p not in sys.path:
        sys.path.append(_p)

import math
import numpy as np
import ml_dtypes

import concourse.bass as bass
import concourse.mybir as mybir
import concourse.tile as tile
from concourse import bacc, bass_utils
from concourse.masks import make_identity

F32 = mybir.dt.float32
F32R = mybir.dt.float32r
BF16 = mybir.dt.bfloat16
AFT = mybir.ActivationFunctionType

P = 128
D = 2048
HD = 64
NJ = HD // 2          # 32 rope freqs
OQ = 512              # q-head dims per core (8 heads * 64)
OKV = 128             # kv-head dims per core (2 heads * 64)
NPAIR = 4             # head pairs per core
DT = D // P           # 16 d-tiles

HEAD_PERM = [0, 4, 1, 5, 2, 6, 3, 7]


def _emit_rope(nc, out_sb, in_ap, cos_ap, sin_ap, nh, tmp_pool):
    """RoPE: out[.., 2j] = x0*c - x1*s ; out[.., 2j+1] = x0*s + x1*c.
    in_ap: [128, nh*64] (PSUM f32); out_sb: [128, nh*64] (SBUF bf16);
    cos_ap/sin_ap: [128, 32] (per s-tile)."""
    w = nh * NJ
    x = in_ap.rearrange("p (h j t) -> p h j t", h=nh, j=NJ, t=2)
    o = out_sb.rearrange("p (h j t) -> p h j t", h=nh, j=NJ, t=2)
    x0, x1 = x[:, :, :, 0], x[:, :, :, 1]
    o0, o1 = o[:, :, :, 0], o[:, :, :, 1]
    c = cos_ap.unsqueeze(1).broadcast_to([P, nh, NJ])
    s = sin_ap.unsqueeze(1).broadcast_to([P, nh, NJ])
    ta = tmp_pool.tile([P, w], F32, tag="rope_ta")
    tb = tmp_pool.tile([P, w], F32, tag="rope_tb")
    ta3 = ta.rearrange("p (h j) -> p h j", h=nh, j=NJ)
    tb3 = tb.rearrange("p (h j) -> p h j", h=nh, j=NJ)
    nc.vector.tensor_mul(ta3, x0, c)
    nc.vector.tensor_mul(tb3, x1, s)
    nc.vector.tensor_sub(o0, ta3, tb3)
    nc.vector.tensor_mul(ta3, x0, s)
    nc.vector.tensor_mul(tb3, x1, c)
    nc.vector.tensor_add(o1, ta3, tb3)


def emit_kernel(nc, tc, ctx, S):
    NSC = S // 512        # s-chunks
    NST = S // P          # s-tiles (global)

    xT_d = nc.dram_tensor("xT", [D, S], BF16, kind="ExternalInput").ap()
    wqT_d = nc.dram_tensor("wqT", [D, OQ], BF16, kind="ExternalInput").ap()
    wkvT_d = nc.dram_tensor("wkvT", [D, 256], BF16, kind="ExternalInput").ap()
    woT_d = nc.dram_tensor("woT", [OQ, D], BF16, kind="ExternalInput").ap()
    cos_d = nc.dram_tensor("cost", [S, NJ], F32, kind="ExternalInput").ap()
    sin_d = nc.dram_tensor("sint", [S, NJ], F32, kind="ExternalInput").ap()
    out_d = nc.dram_tensor("out", [S, D], BF16, kind="ExternalOutput").ap()

    ctx.enter_context(nc.allow_low_precision(reason="bf16/f32r matmuls"))
    const = ctx.enter_context(tc.tile_pool(name="const", bufs=1))
    work = ctx.enter_context(tc.tile_pool(name="work", bufs=2))
    epool = ctx.enter_context(tc.tile_pool(name="epool", bufs=8))
    xTp = ctx.enter_context(tc.tile_pool(name="xTp", bufs=4))
    qTp = ctx.enter_context(tc.tile_pool(name="qTp", bufs=2))
    atp = ctx.enter_context(tc.tile_pool(name="atp", bufs=2))
    psS = ctx.enter_context(tc.tile_pool(name="psS", bufs=2, space="PSUM"))
    psV = ctx.enter_context(tc.tile_pool(name="psV", bufs=1, space="PSUM"))
    psF = ctx.enter_context(tc.tile_pool(name="psF", bufs=2, space="PSUM"))

    idn = const.tile([P, P], BF16)
    make_identity(nc, idn)
    ones_f = const.tile([P, 1], F32)
    nc.any.memset(ones_f[:], 1.0)
    ones64 = const.tile([65, HD], F32R)
    nc.vector.tensor_copy(ones64[:], ones_f[0:65, 0:1].broadcast_to([65, HD]))

    wqT = const.tile([P, DT * OQ], BF16)    # [d_loc, dt*512 + o']
    wkvT = const.tile([P, DT * 256], BF16)  # [d_loc, dt*256 + (k:0-127 | v:128-255)]
    woT = const.tile([P, NPAIR * D], BF16)  # [o'_loc, p*2048 + d]
    kT = const.tile([P, S], BF16)           # [o_kv, s]
    v2 = const.tile([P, NST * 130], BF16)   # [s_loc, g*130 + a*65 + (hd|one)]
    cosr = const.tile([P, NST * NJ], F32)
    sinr = const.tile([P, NST * NJ], F32)

    def emit_proj_load(c, st):
        g = c * 4 + st
        xT = xTp.tile([P, DT * P], BF16, tag="xT", name=f"xT_{g}")
        nc.sync.dma_start(xT[:].rearrange("p (dt s) -> p dt s", dt=DT, s=P),
                          xT_d[:, g * P:(g + 1) * P].rearrange("(dt p) s -> p dt s", p=P))
        return xT

    # weight loads, chunked and ordered so the first projections start early:
    # xT(st0) + first wq/wkv chunks first, wo (first needed ~80us in) last
    xt0 = emit_proj_load(0, 0)
    wq4 = wqT[:].rearrange("p (c4 dt o) -> p c4 dt o", c4=4, dt=4, o=OQ)
    wq4_d = wqT_d.rearrange("(c4 dt p) o -> p c4 dt o", c4=4, p=P)
    wk2 = wkvT[:].rearrange("p (c2 dt o) -> p c2 dt o", c2=2, dt=8, o=256)
    wk2_d = wkvT_d.rearrange("(c2 dt p) o -> p c2 dt o", c2=2, p=P)
    nc.sync.dma_start(wq4[:, 0], wq4_d[:, 0])
    nc.sync.dma_start(wk2[:, 0], wk2_d[:, 0])
    nc.sync.dma_start(cosr[:].rearrange("p (g j) -> p g j", g=NST, j=NJ),
                      cos_d.rearrange("(g p) j -> p g j", p=P))
    nc.sync.dma_start(sinr[:].rearrange("p (g j) -> p g j", g=NST, j=NJ),
                      sin_d.rearrange("(g p) j -> p g j", p=P))
    for i in range(1, 4):
        nc.sync.dma_start(wq4[:, i], wq4_d[:, i])
        if i < 2:
            nc.sync.dma_start(wk2[:, i], wk2_d[:, i])
    wo4 = woT[:].rearrange("p (pp d) -> p pp d", pp=NPAIR, d=D)
    wo4_d = woT_d.rearrange("(pp o) d -> o pp d", o=P)
    for i in range(NPAIR):
        nc.sync.dma_start(wo4[:, i], wo4_d[:, i])

    # ones columns of v2 (positions i*65 + 64)
    v2ones = v2[:].rearrange("p (i c) -> p i c", i=2 * NST, c=65)[:, :, 64]
    nc.vector.tensor_copy(v2ones, ones_f[:, 0:1].broadcast_to([P, 2 * NST]))

    def transpose_pair(srcs, dst):
        """Transpose 1-2 [128,128] blocks through one PSUM slot; dst is a
        [128, len(srcs), 128] strided view evacuated with a single copy.
        Uses a regular matmul against the identity (out = src.T @ I) rather
        than transpose-mode: same result, cheaper, and it counts as PE
        activity for the HAM clock gate (transpose-mode does not)."""
        tp = psF.tile([P, 2 * P], F32, tag="fr")
        for i, src in enumerate(srcs):
            nc.tensor.matmul(tp[:, i * P:(i + 1) * P], src, idn[:])
        n = len(srcs)
        nc.vector.tensor_copy(dst, tp[:, 0:n * P].rearrange("o (a s) -> o a s", a=n))

    def emit_proj(c, qT, st, xT):
        g = c * 4 + st
        cos_ap = cosr[:, g * NJ:(g + 1) * NJ]
        sin_ap = sinr[:, g * NJ:(g + 1) * NJ]
        # q projection [s,o'] and rope (bf16 out)
        qp = psF.tile([P, OQ], F32, tag="fr")
        for dt in range(DT):
            nc.tensor.matmul(qp[:], xT[:, dt * P:(dt + 1) * P],
                             wqT[:, dt * OQ:(dt + 1) * OQ],
                             start=(dt == 0), stop=(dt == DT - 1))
        qr = work.tile([P, OQ], BF16, tag="qr")
        _emit_rope(nc, qr[:], qp[:], cos_ap, sin_ap, 8, work)
        # kv projection [s, k(128)|v(128)] and rope on k part
        kvp = psF.tile([P, 256], F32, tag="fr")
        for dt in range(DT):
            nc.tensor.matmul(kvp[:], xT[:, dt * P:(dt + 1) * P],
                             wkvT[:, dt * 256:(dt + 1) * 256],
                             start=(dt == 0), stop=(dt == DT - 1))
        kr = work.tile([P, OKV], BF16, tag="kr")
        _emit_rope(nc, kr[:], kvp[:, 0:OKV], cos_ap, sin_ap, 2, work)
        # v -> v2 (split the two kv heads around the ones columns)
        v_src = kvp[:, OKV:256].rearrange("p (a x) -> p a x", a=2, x=HD)
        v_dst = v2[:, g * 130:(g + 1) * 130].rearrange("p (a x) -> p a x", a=2, x=65)[:, :, 0:HD]
        nc.vector.tensor_copy(v_dst, v_src)
        # transposes q -> qT, k -> kT (PE transpose + DVE evacuate, 2 blocks
        # per PSUM slot acquisition)
        qT4 = qT[:].rearrange("o (p s) -> o p s", p=NPAIR)
        for pa in (0, 2):
            transpose_pair([qr[:, p * P:(p + 1) * P] for p in (pa, pa + 1)],
                           qT4[:, pa:pa + 2, st * P:(st + 1) * P])
        transpose_pair([kr[:]],
                       kT[:, g * P:(g + 1) * P].unsqueeze(1))

    def emit_attn_pair(c, pg, pp, attnT, qT):
        """Attention for head-pair p = pg*2+pp of chunk c."""
        p = pg * 2 + pp
        NJT = 4 * (c + 1)
        pv = psV.tile([65, 1024], F32, tag="pv", name=f"pv_{c}_{pg}_{pp}")
        for j in range(NJT):
            # causal: only columns q >= j*128 - c*512 within the chunk are live
            vs = max(0, (j - 4 * c) * P)
            w = 512 - vs
            sc2 = psS.tile([P, 1024], F32, tag="sc")
            nc.tensor.matmul(sc2[:, vs:512], kT[0:HD, j * P:(j + 1) * P],
                             qT[0:HD, p * 512 + vs:(p + 1) * 512])
            nc.tensor.matmul(sc2[:, 512 + vs:1024], kT[HD:P, j * P:(j + 1) * P],
                             qT[HD:P, p * 512 + vs:(p + 1) * 512])
            e2 = epool.tile([P, 1024], BF16, tag="e")
            if vs:
                sc_v = sc2[:].rearrange("p (h q) -> p h q", h=2, q=512)[:, :, vs:512]
                e_v = e2[:].rearrange("p (h q) -> p h q", h=2, q=512)[:, :, vs:512]
                nc.scalar.activation(e_v, sc_v, AFT.Exp, scale=1.0 / 8.0)
            else:
                nc.scalar.activation(e2[:], sc2[:], AFT.Exp, scale=1.0 / 8.0)
            if j >= 4 * c:  # diagonal block: zero where k_glob > q_glob
                for half in range(2):
                    nc.gpsimd.affine_select(
                        out=e2[:, half * 512 + vs:(half + 1) * 512],
                        in_=e2[:, half * 512 + vs:(half + 1) * 512],
                        compare_op=mybir.AluOpType.is_ge, fill=0.0,
                        base=c * 512 + vs - j * P, channel_multiplier=-1,
                        pattern=[[1, w]])
            nc.tensor.matmul(pv[:, vs:512], v2[:, j * 130: j * 130 + 65],
                             e2[:, vs:512],
                             start=(j == 0), stop=(j == NJT - 1), skip_group_check=True)
            nc.tensor.matmul(pv[:, 512 + vs:1024],
                             v2[:, j * 130 + 65: (j + 1) * 130],
                             e2[:, 512 + vs:1024],
                             start=(j == 0), stop=(j == NJT - 1), skip_group_check=True)
        # evacuate the accumulator to SBUF immediately so the single PV PSUM
        # slot frees for the next pair's j-loop; normalize runs from SBUF.
        # bf16 so the normalize multiplies hit the DVE 2x perf mode.
        pvs = work.tile([65, 1024], BF16, tag="pvs")
        nc.vector.tensor_copy(pvs[:], pv[:])
        # normalize: attnT rows = outT * Zinv ; Z sits in row 64.
        # Zinv = exp(-ln Z): ln+exp live in one ACT table set with the
        # scores' exp, so no table reloads; one batched [1,1024] op pair.
        lnz = work.tile([65, 1024], F32, tag="lnz")
        nc.scalar.activation(lnz[64:65, :], pvs[64:65, :], AFT.Ln)
        zi = work.tile([65, 1024], F32R, tag="zi")
        nc.scalar.activation(zi[64:65, :], lnz[64:65, :], AFT.Exp, scale=-1.0)
        for half in range(2):
            bc = psF.tile([HD, 512], F32, tag="fr")
            nc.tensor.matmul(bc[:], ones64[64:65, :],
                             zi[64:65, half * 512:(half + 1) * 512])
            bcs = work.tile([HD, 512], BF16, tag="bc")
            nc.vector.tensor_copy(bcs[:], bc[:])
            if half == 0:
                nc.vector.tensor_mul(attnT[0:HD, p * 512:(p + 1) * 512],
                                     pvs[0:HD, 0:512], bcs[:])
            else:
                tmpb = work.tile([HD, 512], BF16, tag="tmpb", bufs=4)
                nc.vector.tensor_mul(tmpb[:], pvs[0:HD, 512:1024], bcs[:])
                # partition shift 0:64 -> 64:128 via sbuf-sbuf DMA
                nc.sync.dma_start(attnT[HD:P, p * 512:(p + 1) * 512], tmpb[:])

    def emit_final_st(c, attnT, st, tail=False):
        # res[s, d] = sum_p attnT_p.T @ woT_p, one s-tile row
        for dc in range(4):
            rp = psF.tile([P, 512], F32, tag="fr")
            for p in range(NPAIR):
                nc.tensor.matmul(rp[:], attnT[:, p * 512 + st * P: p * 512 + (st + 1) * P],
                                 woT[:, p * D + dc * 512: p * D + (dc + 1) * 512],
                                 start=(p == 0), stop=(p == NPAIR - 1), skip_group_check=True)
            rs = work.tile([P, 512], BF16, tag="rs")
            if tail and dc % 2 == 0:
                # drain phase: ACT is idle, split evacuations across engines
                nc.scalar.copy(rs[:], rp[:])
            else:
                nc.vector.tensor_copy(rs[:], rp[:])
            nc.sync.dma_start(out_d[(c * 4 + st) * P:(c * 4 + st + 1) * P,
                                    dc * 512:(dc + 1) * 512], rs[:])

    # ---- main loop: proj for chunk c+1 is interleaved between the
    # ACT-bound attention pair-groups of chunk c so the PE always has
    # independent matmul work; final(c) trails into chunk c+1. ----
    qT_cur = qTp.tile([P, NPAIR * 512], BF16, tag="qT", name="qT_0")
    xts = [xt0] + [emit_proj_load(0, st) for st in range(1, 4)]
    for st in range(4):
        emit_proj(0, qT_cur, st, xts[st])
    pending_final = None
    for c in range(NSC):
        qT = qT_cur
        if c + 1 < NSC:
            qT_cur = qTp.tile([P, NPAIR * 512], BF16, tag="qT", name=f"qT_{c+1}")
            xts = [emit_proj_load(c + 1, 0)]
        attnT = atp.tile([P, NPAIR * 512], BF16, tag="attnT")
        k = 0
        for pg in range(2):
            for pp in range(2):
                emit_attn_pair(c, pg, pp, attnT, qT)
                if c + 1 < NSC:
                    if k + 1 < 4:
                        xts.append(emit_proj_load(c + 1, k + 1))
                    emit_proj(c + 1, qT_cur, k, xts[k])
                if pending_final is not None:
                    # final matmuls of the previous chunk: dependency-free PE
                    # fill spread across this chunk's ACT-bound pair groups
                    emit_final_st(pending_final[0], pending_final[1], k)
                k += 1
        pending_final = (c, attnT)
    for st in range(4):
        emit_final_st(pending_final[0], pending_final[1], st, tail=True)


_NC_CACHE = {}


def _pin_exp_ln_table_set():
    """Make the ACT-table-load pass resolve both Exp and Ln to the one set
    that contains them both (natural_log_exp_and_others). The default
    first-containing-set choice alternates exp_and_others / natural_log per
    activation, inserting a ~1.3us table reload before every softmax
    normalization. Only the advertised membership used for set *selection*
    is filtered; set indices stay canonical, so the runtime tables match."""
    if getattr(bacc, "_exp_ln_pinned", False):
        return
    real = bacc.get_activation_tables

    def pinned(arch):
        tables = dict(real(arch))
        both = {AFT.Exp, AFT.Ln}
        for name in list(tables):
            if name != "natural_log_exp_and_others" and (tables[name] & both):
                tables[name] = tables[name] - both
        return tables

    bacc.get_activation_tables = pinned
    bacc._exp_ln_pinned = True


def build(S=2048):
    if S in _NC_CACHE:
        return _NC_CACHE[S]
    from contextlib import ExitStack
    _pin_exp_ln_table_set()
    nc = bacc.Bacc("TRN2", target_bir_lowering=False, debug=False, num_devices=8)
    with tile.TileContext(nc) as tc, ExitStack() as ctx:
        emit_kernel(nc, tc, ctx, S)
    nc.compile()
    _NC_CACHE[S] = nc
    return nc


def shard_inputs(x, theta, wq, wk, wv, wo, S=2048):
    """Returns in_maps for 8 cores: core = b*4 + g. Pure layout prep."""
    cost = np.cos(theta[:S]).astype(np.float32)
    sint = np.sin(theta[:S]).astype(np.float32)
    in_maps = []
    for core in range(8):
        b, g = core // 4, core % 4
        wq_g = wq[g * 512:(g + 1) * 512].reshape(8, HD, D)[HEAD_PERM].reshape(512, D)
        wo_g = wo[:, g * 512:(g + 1) * 512].reshape(D, 8, HD)[:, HEAD_PERM].reshape(D, 512)
        wkv_g = np.concatenate([wk[g * 128:(g + 1) * 128], wv[g * 128:(g + 1) * 128]], axis=0)
        bf = ml_dtypes.bfloat16
        in_maps.append({
            "xT": np.ascontiguousarray(x[b, :S].T).astype(bf),
            "wqT": np.ascontiguousarray(wq_g.T).astype(bf),
            "wkvT": np.ascontiguousarray(wkv_g.T).astype(bf),
            "woT": np.ascontiguousarray(wo_g.T).astype(bf),
            "cost": cost,
            "sint": sint,
        })
    return in_maps


def run_on_hw(inputs, S=2048, trace=False):
    nc = build(S)
    in_maps = shard_inputs(inputs["x"], inputs["theta"], inputs["wq"],
                           inputs["wk"], inputs["wv"], inputs["wo"], S=S)
    res = bass_utils.run_bass_kernel_spmd(nc, in_maps, core_ids=list(range(8)),
                                          trace=trace)
    parts = [res.results[c]["out"].astype(np.float32) for c in range(8)]
    out = np.stack([parts[0] + parts[1] + parts[2] + parts[3],
                    parts[4] + parts[5] + parts[6] + parts[7]], axis=0)
    return out, res


def kernel(x, theta, mask, wq, wk, wv, wo):
    out, _ = run_on_hw({"x": np.asarray(x, np.float32), "theta": np.asarray(theta, np.float32),
                        "wq": np.asarray(wq, np.float32), "wk": np.asarray(wk, np.float32),
                        "wv": np.asarray(wv, np.float32), "wo": np.asarray(wo, np.float32)})
    return out
